# revision 15
# baseline (speedup 1.0000x reference)
"""Trainium2 Bass kernel for BasicDMPNN (gnn_message_passing).

Strategy (v2 — matmul segment-sum, no dma_scatter_add):
  - Nodes are partitioned contiguously across 8 cores (12500 each); every
    edge is owned by the core that owns its dst node, so the segment-sum
    is core-local.
  - Edge MLPs fold into tiny tables:
      msg_0[e] = relu(Ci[code[e]]),  msg_r[e] = relu(Cu[code[e]] + aggW[src[e]])
    with code[e] = 4*x[src[e]] + edge_attr[e], aggW = agg @ Wu2.
  - Per round: each core computes aggW for its slice on PE, AllGather
    makes the full [100352, 64] table visible, the edge pass gathers
    aggW[src] via SWDGE dma_gather (int16 idx => 4 src groups of 25088
    rows), adds the streamed base (bf16) and applies relu.
  - The dst segment-sum runs on PE instead of dma_scatter_add: edges are
    dst-sorted into a fixed slot grid (128-edge slots per 1024-node
    cell, slot count = max over cores so the SPMD program is shared);
    per slot a one-hot S[e, j] = (dst_local[e] - 128*w == j) is built on
    DVE (iota vs per-partition scalar) and msg^T @ S accumulates into a
    PSUM window tile [64 feat, 128 nodes]; window tiles flush into an
    SBUF slab aggT [64, 12544] (copy on first group, add after).
  - Node stage reads the slab directly: aggW tile = matmul(slabT_w, Wu2),
    one DMA to DRAM, AllGather.  No DRAM agg, no accumulators, no
    combine pass.
  - Molecule readout: after the last edge pass the slab is transposed
    (PE) to node-major and summed into a per-core 768-mol PSUM window
    via the same on-chip one-hot trick (batch is sorted), AllGather of
    the [64, 768] partials, combine at static per-core offsets, MLP head.
"""

import os

import numpy as np

import concourse.bacc as bacc
import concourse.bass as bass
import concourse.mybir as mybir
import concourse.tile as tile
from concourse import bass_utils
from concourse.masks import make_identity

N_CORES = 8
N_NODES = 100000
NPC = 12500          # nodes per core
NPCP = 12544         # padded node slice (98 * 128)
CHUNK_ROWS = 2 * NPCP  # 25088 rows per src group of the allgathered table
FULL_ROWS = N_CORES * NPCP  # 100352
MOLS = 2048
MOLW = 512           # per-core mol window (4 cells of 128)
MOLP = MOLS + MOLW   # molT padded with scratch cols (per-core windows may
                     # start within 128 of the core's first mol)
MSG = 64
BLOCK = 1024
TB = BLOCK // 128    # 8 slots per block
NT = NPCP // 128     # 98 node windows per core
CELL = 1024          # nodes per grid cell (8 windows)
NCELL = (NPCP + CELL - 1) // CELL  # 13
ROUNDS = int(os.environ.get("DMPNN_ROUNDS", "4"))
REPEAT = int(os.environ.get("DMPNN_REPEAT", "1"))  # whole-kernel repeats (bench)
F32 = mybir.dt.float32
BF16 = mybir.dt.bfloat16
I16 = mybir.dt.int16

_CACHE = {}


def _wrap16(idx, nblocks):
    """[nblocks*B] int -> [nblocks, 128, B//16] int16: index i of a block
    sits at [i % 16, i // 16], replicated across the 8 Q7 core groups."""
    b = idx.size // nblocks
    v = np.transpose(idx.reshape(nblocks, b // 16, 16), (0, 2, 1)).astype(np.int16)
    return np.tile(v, (1, 8, 1))


def _prep(inputs):
    x = np.asarray(inputs["x"]).astype(np.int64)
    ea = np.asarray(inputs["edge_attr"]).astype(np.int64)
    ei = np.asarray(inputs["edge_index"]).astype(np.int64)
    batch = np.asarray(inputs["batch"]).astype(np.int64)
    atom_table = np.asarray(inputs["atom_table"], np.float32)
    bond_table = np.asarray(inputs["bond_table"], np.float32)
    Wi = np.asarray(inputs["Wi"], np.float32)
    bi = np.asarray(inputs["bi"], np.float32)
    Wu = np.asarray(inputs["Wu"], np.float32)
    bu = np.asarray(inputs["bu"], np.float32)

    src, dst = ei[0], ei[1]
    a_i = atom_table @ Wi[:64]
    b_i = bond_table @ Wi[64:80]
    a_u = atom_table @ Wu[:64]
    b_u = bond_table @ Wu[64:80]
    Ci = (a_i[:, None, :] + b_i[None, :, :] + bi).reshape(476, 64)
    Cu = (a_u[:, None, :] + b_u[None, :, :] + bu).reshape(476, 64)
    cc = np.zeros((476, 128), np.float32)
    cc[:, :64] = Ci
    cc[:, 64:] = Cu

    code = 4 * x[src] + ea
    gidx = src // (2 * NPC)                      # src group 0..3
    grow16 = (src // NPC) * NPCP + (src % NPC) - gidx * CHUNK_ROWS
    owner = dst // NPC
    dloc = dst - owner * NPC

    # --- slot grid: per (group, cell) slot count = max over cores -------
    per = {}
    cnt = np.zeros((N_CORES, 4, NCELL), np.int64)
    for c in range(N_CORES):
        mc = owner == c
        for g in range(4):
            m = mc & (gidx == g)
            sel = np.nonzero(m)[0]
            dl = dloc[sel]
            order = np.argsort(dl, kind="stable")
            sel, dl = sel[order], dl[order]
            cells = dl // CELL
            bc = np.bincount(cells, minlength=NCELL)
            cnt[c, g] = bc
            ofs = np.concatenate([[0], np.cumsum(bc)])
            per[c, g] = [(sel[ofs[i]:ofs[i + 1]], dl[ofs[i]:ofs[i + 1]])
                         for i in range(NCELL)]
    maxslot = np.zeros((4, NCELL), np.int64)
    for g in range(4):
        maxslot[g] = np.ceil(cnt[:, g, :].max(axis=0) / 128).astype(np.int64)
    slots_g = maxslot.sum(axis=1)
    nb = [int(np.ceil(s / TB)) for s in slots_g]
    nbtot = sum(nb)

    # slot -> (block, t) and slot -> cell maps, group-major
    slot_cell = []   # per global slot: (g, cell) or None for pad
    block_g = []
    for g in range(4):
        for cell in range(NCELL):
            slot_cell += [(g, cell)] * int(maxslot[g, cell])
        slot_cell += [None] * (nb[g] * TB - int(slots_g[g]))
        block_g += [g] * nb[g]
    assert len(slot_cell) == nbtot * TB

    # --- fill per-core slot data + per-slot window ranges ---------------
    s16 = np.zeros((N_CORES, nbtot * BLOCK), np.int64)
    c16 = np.zeros((N_CORES, nbtot * BLOCK), np.int64)
    draw = np.full((N_CORES, nbtot, TB, 128), -(1 << 20), np.int64)
    wlo = np.full(nbtot * TB, 1 << 30, np.int64)
    whi = np.full(nbtot * TB, -1, np.int64)
    slot_of = {}
    si = 0
    for g in range(4):
        for cell in range(NCELL):
            slot_of[g, cell] = si
            si += int(maxslot[g, cell])
        si += nb[g] * TB - int(slots_g[g])
    for c in range(N_CORES):
        for g in range(4):
            for cell in range(NCELL):
                sel, dl = per[c, g][cell]
                s0 = slot_of[g, cell]
                for j in range(0, len(sel), 128):
                    s = s0 + j // 128
                    b, t = divmod(s, TB)
                    n = min(128, len(sel) - j)
                    pos = b * BLOCK + t * 128
                    s16[c, pos:pos + n] = grow16[sel[j:j + n]]
                    c16[c, pos:pos + n] = code[sel[j:j + n]]
                    draw[c, b, t, :n] = dl[j:j + n]
                    wlo[s] = min(wlo[s], int(dl[j] // 128))
                    whi[s] = max(whi[s], int(dl[j + n - 1] // 128))

    # --- per-slot segment schedule + flush/start maps -------------------
    # window touch = the full [wlo, whi] range (matmuls are emitted for
    # gap windows too); first/last touch are per group.
    MAXSEG = 4
    first = {}
    last = {}
    for s in range(nbtot * TB):
        if whi[s] < 0:
            continue
        assert whi[s] - wlo[s] + 1 <= MAXSEG, (s, wlo[s], whi[s])
        g = slot_cell[s][0]
        for w in range(int(wlo[s]), int(whi[s]) + 1):
            if (g, w) not in first:
                first[g, w] = s
            last[g, w] = s
    wfirst_g = {}
    for (g, w) in first:
        wfirst_g[w] = min(wfirst_g.get(w, 9), g)
    assert len(wfirst_g) == NT, (len(wfirst_g), NT)
    sched = []
    for b in range(nbtot):
        segs = []
        for t in range(TB):
            s = b * TB + t
            if whi[s] < 0:
                segs.append((t, 0, 0, (), (), ()))
                continue
            g = slot_cell[s][0]
            w0, n = int(wlo[s]), int(whi[s] - wlo[s] + 1)
            st = tuple(first[g, w0 + k] == s for k in range(n))
            sp = tuple(last[g, w0 + k] == s for k in range(n))
            fl = tuple((w, wfirst_g[w] == g) for w in range(w0, w0 + n)
                       if last[g, w] == s)
            segs.append((t, w0, n, st, sp, fl))
        sched.append(tuple(segs))
    sched = tuple(sched)
    # sanity: every (g, w) flushed exactly once
    assert len(first) == len(last)

    # dstoff int16: draw - 128*w0(slot); sentinel -> -512
    w0_arr = np.where(whi >= 0, wlo, 0).reshape(nbtot, TB)
    doff = draw - 128 * w0_arr[None, :, :, None]
    doff = np.where(draw < 0, -512, doff)
    assert doff.max() < 32767 and doff.min() >= -32768

    idx_all = np.zeros((N_CORES, nbtot, 128, 72), np.int16)
    idx0 = np.zeros((N_CORES, nbtot, 128, 64), np.int16)
    for c in range(N_CORES):
        idx_all[c, :, :, 0:64] = _wrap16(s16[c], nbtot)
        idx_all[c, :, :, 64:72] = np.transpose(doff[c], (0, 2, 1)).astype(np.int16)
        idx0[c] = _wrap16(c16[c], nbtot)

    # --- molecule stage -------------------------------------------------
    molw0 = []
    mcell_lo = np.full(NT, 1 << 30, np.int64)
    mcell_hi = np.full(NT, -1, np.int64)
    boff = np.zeros((N_CORES, 128, NT), np.float32)
    braw = np.zeros((N_CORES, NT, 128), np.int64)
    for c in range(N_CORES):
        bl = batch[c * NPC:(c + 1) * NPC]
        w0 = int((bl[0] // 128) * 128)
        molw0.append(w0)
        assert bl[-1] - w0 < MOLW, (c, bl[0], bl[-1], w0)
        blp = np.full(NPCP, -(1 << 20), np.int64)
        blp[:NPC] = bl - w0
        braw[c] = blp.reshape(NT, 128)
        mc = blp.reshape(NT, 128) // 128
        for t in range(NT):
            v = mc[t][mc[t] >= 0]
            mcell_lo[t] = min(mcell_lo[t], int(v.min()))
            mcell_hi[t] = max(mcell_hi[t], int(v.max()))
    assert (mcell_hi - mcell_lo + 1).max() <= 4
    mfirst = {}
    mlast = {}
    for t in range(NT):
        for k in range(int(mcell_lo[t]), int(mcell_hi[t]) + 1):
            if k not in mfirst:
                mfirst[k] = t
            mlast[k] = t
    msched = []
    for t in range(NT):
        c0, n = int(mcell_lo[t]), int(mcell_hi[t] - mcell_lo[t] + 1)
        sp = tuple(mlast[c0 + k] == t for k in range(n))
        msched.append((c0, n, sp))
    msched = tuple(msched)
    mcells_unused = tuple(k for k in range(MOLW // 128) if k not in mlast)
    for c in range(N_CORES):
        bo = braw[c] - 128 * mcell_lo[:, None]
        bo = np.where(braw[c] < -(1 << 19), -512, bo)
        boff[c] = bo.T.astype(np.float32)

    tables = dict(
        cc_cat=cc,
        wu2=np.ascontiguousarray(Wu[80:144]),
        w1=np.asarray(inputs["W1"], np.float32),
        w2=np.asarray(inputs["W2"], np.float32),
        b1=np.asarray(inputs["b1"], np.float32).reshape(128, 1),
        b2=np.full((128, 1), float(np.asarray(inputs["b2"]).reshape(-1)[0]),
                   np.float32),
    )
    per_core_inputs = []
    for c in range(N_CORES):
        m = dict(tables)
        m["idx_all"] = idx_all[c]
        m["idx0"] = idx0[c]
        m["boff"] = boff[c]
        per_core_inputs.append(m)
    key = (tuple(nb), tuple(block_g), sched, msched, mcells_unused,
           tuple(molw0))
    return per_core_inputs, key, nbtot, tuple(molw0)


def _build(key, nbtot, molw0):
    nb, block_g, sched, msched, mcells_unused, _ = key
    nc = bacc.Bacc(
        "TRN2", target_bir_lowering=False, debug=False, num_devices=N_CORES,
        num_swdge_queues=4,
    )
    t_cc = nc.dram_tensor("cc_cat", [476, 128], F32, kind="ExternalInput")
    t_wu2 = nc.dram_tensor("wu2", [64, 64], F32, kind="ExternalInput")
    t_w1 = nc.dram_tensor("w1", [64, 128], F32, kind="ExternalInput")
    t_w2 = nc.dram_tensor("w2", [128, 1], F32, kind="ExternalInput")
    t_b1 = nc.dram_tensor("b1", [128, 1], F32, kind="ExternalInput")
    t_b2 = nc.dram_tensor("b2", [128, 1], F32, kind="ExternalInput")
    t_idx = nc.dram_tensor("idx_all", [nbtot, 128, 72], I16, kind="ExternalInput")
    t_idx0 = nc.dram_tensor("idx0", [nbtot, 128, 64], I16, kind="ExternalInput")
    t_boff = nc.dram_tensor("boff", [128, NT], F32, kind="ExternalInput")
    t_out = nc.dram_tensor("out", [2048], F32, kind="ExternalOutput")

    with tile.TileContext(nc) as tc:
        with (
            tc.tile_pool(name="dram", bufs=1, space="DRAM") as dram,
            tc.tile_pool(name="const", bufs=1) as constp,
            tc.tile_pool(name="slabp", bufs=1) as slabp,
            tc.tile_pool(name="sb", bufs=int(os.environ.get("DMPNN_SBUFS", "6"))) as sb,
            tc.tile_pool(name="nsb", bufs=1) as nsb,
            tc.tile_pool(name="psw", bufs=4, space="PSUM") as psw,
            tc.tile_pool(name="psn", bufs=2, space="PSUM") as psn,
            tc.tile_pool(name="psm", bufs=1, space="PSUM") as psm,
        ):
            base_u = dram.tile([nbtot, 128, TB * MSG], BF16)
            aggw_s = dram.tile([NPCP, MSG], F32)
            aggw_fr = []
            for rr in range(ROUNDS):
                afr = dram.tile([FULL_ROWS, MSG], F32, tag=f"aggwf_r{rr}",
                                name=f"aggwf_r{rr}")
                aggw_fr.append(afr)
            molg_in = dram.tile([64, MOLW], F32)
            molg_out = dram.tile([N_CORES * 64, MOLW], F32)

            ident = constp.tile([128, 128], F32)
            make_identity(nc, ident[:])
            wu2 = constp.tile([64, 64], F32)
            nc.sync.dma_start(wu2[:], t_wu2[:, :])
            w1 = constp.tile([64, 128], F32)
            nc.sync.dma_start(w1[:], t_w1[:, :])
            w2 = constp.tile([128, 1], F32)
            nc.sync.dma_start(w2[:], t_w2[:, :])
            b1 = constp.tile([128, 1], F32)
            nc.sync.dma_start(b1[:], t_b1[:, :])
            b2v = constp.tile([128, 1], F32)
            nc.sync.dma_start(b2v[:], t_b2[:, :])
            boff_sb = constp.tile([128, NT], F32)
            nc.sync.dma_start(boff_sb[:], t_boff[:, :])
            iota_i = constp.tile([128, 512], mybir.dt.int32)
            nc.gpsimd.iota(iota_i[:], pattern=[[1, 512]], base=0,
                           channel_multiplier=0)
            iotaW = constp.tile([128, 512], F32)
            nc.vector.tensor_copy(iotaW[:], iota_i[:])
            zeroS = constp.tile([128, 128], F32)
            nc.vector.memset(zeroS[:], 0.0)

            # aggT slab: [64 feat, 12544 nodes], persists across one round
            slab = slabp.tile([64, NPCP], F32)
            # node-major staging (aggW tiles / final agg transpose)
            nmaj = slabp.tile([128, NT * MSG], F32)

            def edge_pass(rnd):
                win_tiles = {}
                for b in range(nbtot):
                    g = block_g[b]
                    idxt = sb.tile([128, 72], I16, tag="idxt")
                    nc.sync.dma_start(idxt[:], t_idx[b])
                    dstf = sb.tile([128, 8], F32, tag="dstf")
                    nc.vector.tensor_copy(dstf[:], idxt[:, 64:72])
                    msg = sb.tile([128, TB, MSG], BF16, tag="msg")
                    if rnd == 0:
                        idx0t = sb.tile([128, 64], I16, tag="idx0t")
                        nc.sync.dma_start(idx0t[:], t_idx0[b])
                        gath0 = sb.tile([128, TB, 128], F32, tag="gath0")
                        nc.gpsimd.dma_gather(
                            gath0[:, :, :], t_cc[:, :], idx0t[:, 0:64],
                            BLOCK, BLOCK, 128, queue_num=b % 4,
                        )
                        nc.scalar.activation(
                            msg[:, :, :], gath0[:, :, 0:64],
                            mybir.ActivationFunctionType.Relu,
                        )
                        baseb = sb.tile([128, TB, MSG], BF16, tag="baseb")
                        nc.vector.tensor_copy(baseb[:, :, :], gath0[:, :, 64:128])
                        nc.scalar.dma_start(
                            base_u[b].rearrange("p (a c) -> p a c", c=MSG),
                            baseb[:, :, :],
                        )
                    else:
                        gath = sb.tile([128, TB, MSG], F32, tag="gath")
                        nc.gpsimd.dma_gather(
                            gath[:, :, :],
                            aggw_fr[rnd - 1][g * CHUNK_ROWS:(g + 1) * CHUNK_ROWS, :],
                            idxt[:, 0:64], BLOCK, BLOCK, MSG, queue_num=b % 4,
                        )
                        bb = sb.tile([128, TB * MSG], BF16, tag="bb")
                        nc.scalar.dma_start(bb[:], base_u[b])
                        summ = sb.tile([128, TB * MSG], F32, tag="summ")
                        nc.vector.tensor_tensor(
                            out=summ[:],
                            in0=gath[:].rearrange("p a c -> p (a c)"),
                            in1=bb[:],
                            op=mybir.AluOpType.add,
                        )
                        nc.scalar.activation(
                            msg[:].rearrange("p a c -> p (a c)"), summ[:],
                            mybir.ActivationFunctionType.Relu,
                        )
                    for (t, w0, nseg, st, sp, fl) in sched[b]:
                        if nseg:
                            S = sb.tile([128, MAXW], BF16, tag="S")
                            nc.vector.tensor_scalar(
                                S[:, 0:nseg * 128], iotaW[:, 0:nseg * 128],
                                dstf[:, t:t + 1], None,
                                mybir.AluOpType.is_equal,
                            )
                            for k in range(nseg):
                                w = w0 + k
                                if st[k]:
                                    wt = psw.tile([64, 128], F32, tag="win",
                                                  space="PSUM")
                                    win_tiles[g, w] = wt
                                else:
                                    wt = win_tiles[g, w]
                                nc.tensor.matmul(
                                    wt[:], msg[:, t, :], S[:, k * 128:(k + 1) * 128],
                                    start=st[k], stop=sp[k],
                                )
                        for w, isfirst in fl:
                            wt = win_tiles.pop((g, w))
                            if isfirst:
                                nc.vector.tensor_copy(
                                    slab[:, w * 128:(w + 1) * 128], wt[:]
                                )
                            else:
                                nc.vector.tensor_tensor(
                                    out=slab[:, w * 128:(w + 1) * 128],
                                    in0=slab[:, w * 128:(w + 1) * 128],
                                    in1=wt[:], op=mybir.AluOpType.add,
                                )
                assert not win_tiles, list(win_tiles)

            def node_stage(rnd):
                for w in range(NT):
                    wp = psn.tile([128, MSG], F32, tag="wp", space="PSUM")
                    nc.tensor.matmul(
                        wp[:], slab[:, w * 128:(w + 1) * 128], wu2[:],
                        start=True, stop=True,
                    )
                    nc.vector.tensor_copy(nmaj[:, w * MSG:(w + 1) * MSG], wp[:])
                nc.sync.dma_start(
                    aggw_s[:].rearrange("(w p) f -> p w f", p=128),
                    nmaj[:].rearrange("p (w f) -> p w f", f=MSG),
                )
                nc.gpsimd.collective_compute(
                    "AllGather", mybir.AluOpType.bypass,
                    replica_groups=[list(range(N_CORES))],
                    ins=[aggw_s[:]], outs=[aggw_fr[rnd][:]],
                )

            def mol_stage():
                for w in range(NT):
                    tp = psn.tile([128, MSG], F32, tag="wp", space="PSUM")
                    nc.tensor.transpose(
                        tp[:], slab[:, w * 128:(w + 1) * 128], ident[0:64, 0:64]
                    )
                    nc.vector.tensor_copy(nmaj[:, w * MSG:(w + 1) * MSG], tp[:])
                molp = psm.tile([64, MOLW], F32, tag="molp", space="PSUM")
                for k in range(MOLW // 128):
                    nc.tensor.matmul(
                        molp[:, k * 128:(k + 1) * 128], nmaj[:, 0:MSG], zeroS[:],
                        start=True, stop=(k in mcells_unused),
                    )
                for t in range(NT):
                    c0, n, sp = msched[t]
                    Sm = sb.tile([128, MAXW], F32, tag="Sm")
                    nc.vector.tensor_scalar(
                        Sm[:, 0:n * 128], iotaW[:, 0:n * 128],
                        boff_sb[:, t:t + 1], None,
                        mybir.AluOpType.is_equal,
                    )
                    for k in range(n):
                        nc.tensor.matmul(
                            molp[:, (c0 + k) * 128:(c0 + k + 1) * 128],
                            nmaj[:, t * MSG:(t + 1) * MSG],
                            Sm[:, k * 128:(k + 1) * 128],
                            start=False, stop=sp[k],
                        )
                molw_sb = nsb.tile([64, MOLW], F32, tag="molw")
                nc.vector.tensor_copy(molw_sb[:], molp[:])
                nc.sync.dma_start(molg_in[:], molw_sb[:])
                nc.gpsimd.collective_compute(
                    "AllGather", mybir.AluOpType.bypass,
                    replica_groups=[list(range(N_CORES))],
                    ins=[molg_in[:]], outs=[molg_out[:]],
                )
                molT = nsb.tile([64, MOLP], F32, tag="molT")
                nc.vector.memset(molT[:], 0.0)
                for c in range(N_CORES):
                    gc = nsb.tile([64, MOLW], F32, tag="molg")
                    nc.sync.dma_start(gc[:], molg_out[c * 64:(c + 1) * 64, :])
                    w0 = molw0[c]
                    nc.vector.tensor_tensor(
                        out=molT[:, w0:w0 + MOLW],
                        in0=molT[:, w0:w0 + MOLW],
                        in1=gc[:], op=mybir.AluOpType.add,
                    )
                # readout: hT = relu(W1^T @ molT + b1); out = hT^T @ W2 + b2
                hT = nsb.tile([128, MOLS], F32, tag="hT")
                for q in range(MOLS // 512):
                    hp = psm.tile([128, 512], F32, tag="hp", space="PSUM")
                    nc.tensor.matmul(
                        hp[:], w1[:], molT[:, q * 512:(q + 1) * 512],
                        start=True, stop=True,
                    )
                    nc.scalar.activation(
                        hT[:, q * 512:(q + 1) * 512], hp[:],
                        mybir.ActivationFunctionType.Relu, bias=b1[:, :1],
                    )
                ot = nsb.tile([128, 16], F32, tag="ot")
                for q in range(16):
                    op_ = psm.tile([128, 512], F32, tag="hp", space="PSUM")
                    nc.tensor.matmul(
                        op_[:, 0:1], hT[:, q * 128:(q + 1) * 128], w2[:],
                        start=True, stop=True,
                    )
                    nc.vector.tensor_copy(ot[:, q:q + 1], op_[:, 0:1])
                ob = nsb.tile([128, 16], F32, tag="ob")
                nc.vector.tensor_scalar_add(ob[:], ot[:], b2v[:, :1])
                nc.sync.dma_start(t_out[:].rearrange("(t p) -> p t", p=128), ob[:])

            MAXW = 512
            for _rep in range(REPEAT):
                for rnd in range(ROUNDS):
                    edge_pass(rnd)
                    node_stage(rnd)
                edge_pass(ROUNDS)
                mol_stage()

    nc.compile()
    return nc


def kernel(**inputs):
    per_core_inputs, key, nbtot, molw0 = _prep(inputs)
    if key not in _CACHE:
        _CACHE[key] = _build(key, nbtot, molw0)
    nc = _CACHE[key]
    res = bass_utils.run_bass_kernel_spmd(
        nc, per_core_inputs, core_ids=list(range(N_CORES))
    )
    return np.asarray(res.results[0]["out"], np.float32)
